# revision 7
# baseline (speedup 1.0000x reference)
"""Trainium2 Bass kernel for nn_CDGMLinear (2-layer graph-learning GNN).

Math per layer (reference):
    g    = relu(x @ gl_w + gl_b)                      # [N, L]
    dist = sq[:,None] + sq[None,:] - 2 g g^T          # [N, N]
    adj  = sigmoid((1+temp) * (-dist) + (5+theta))    # [N, N]
    gnn  = x @ gnn_w + gnn_b                          # [N, D]
    out  = (adj @ gnn) / rowsum(adj)
Layer 1 output gets relu; then out head: softmax(x @ out_w + out_b).

Sharding: row-block over 8 cores (block B = N/8 rows of the adjacency per
core).  Each core computes adj^T tiles [j_tile=128, i in its block] so the
message matmul contracts j on the partition axis.  The N x N matrix never
touches HBM.  One bf16 AllGather moves layer-1 activations between layers.

Precision scheme: all O(N^2) matmuls in bf16.  The diagonal of adj carries
~99.9% of the row mass and is deterministically sigmoid(5+theta) (dist_ii
cancels exactly because sq_i is computed from the same bf16 g values the
PE contracts).  The bf16 quantization of that diagonal is corrected with a
rank-preserving fp32 term:  msg += sigth * gnn_f32 - bf16(sigth) * gnn_bf16
(the bf16 gnn block values are re-derived bit-exactly), and rowsum gets the
scalar correction sigth - bf16(sigth).
"""
import numpy as np
import ml_dtypes

import concourse.bass as bass
import concourse.bacc as bacc
import concourse.tile as tile
import concourse.mybir as mybir
from concourse.bass_utils import run_bass_kernel_spmd

F32 = mybir.dt.float32
BF16 = mybir.dt.bfloat16
Act = mybir.ActivationFunctionType
Alu = mybir.AluOpType
AX = mybir.AxisListType.X

N = 16384
D = 128
L = 64
NCORES = 8
B = N // NCORES          # 2048 rows per core
JT = N // 128            # 128 j-tiles
ICH = 1024               # i-chunk width of the main loop
NIC = B // ICH           # 2 chunks
NOUT = 10

_NC_CACHE = {}


def _layer_prep(nc, sb, misc, x_bf, xr_bf, xr_f32, w, lidx):
    """Emit projection/prep for one layer.  Returns dict of SBUF APs."""
    bcb = misc.tile([128, 512], F32, name=f"bcb{lidx}", tag="z")
    for q in range(4):
        nc.tensor.matmul(bcb[:, q * 128:(q + 1) * 128], w["ones1f"][:],
                         w["gnnbrow"][:, :], start=True, stop=True)
    bcb_sb = sb.tile([128, 512], F32, name=f"bcb_sb{lidx}", tag="bcb_sb")
    nc.vector.tensor_copy(bcb_sb[:], bcb[:])

    # --- block-side moving operand aug_mov [66, B]: rows 0:64 = bf16(2t * g)
    # so the sigmoid's scale is the constant 1.0 (cheaper ACT instruction).
    aug_mov = sb.tile([66, B], BF16, name=f"aug_mov{lidx}", tag="aug_mov")
    gr = sb.tile([64, B], BF16, name=f"gr{lidx}", tag="gr")
    gsqr = sb.tile([64, B], F32, name=f"gsqr{lidx}", tag="gsqr")
    for bc in range(B // 512):
        cs = slice(bc * 512, (bc + 1) * 512)
        gp3 = misc.tile([64, 512], F32, name=f"gp3{lidx}_{bc}", tag="z")
        nc.tensor.matmul(gp3[:], w["wgl_bf"][:], xr_bf[:, cs], start=True, stop=True)
        if bc % 2 == 0:
            nc.scalar.activation(gr[:, cs], gp3[:], Act.Relu,
                                 bias=w["glb"][0:64, :])
        else:
            nc.vector.tensor_scalar(gr[:, cs], gp3[:], w["glb"][0:64, :], 0.0,
                                    Alu.add, Alu.max)
        nc.scalar.mul(aug_mov[0:64, cs], gr[:, cs], w["twot"][0:64, :])
        # exactly the products the PE's diagonal contraction computes
        nc.vector.tensor_tensor(gsqr[:, cs], gr[:, cs], aug_mov[0:64, cs],
                                Alu.mult)
    # sq_i row: -sq_i/2 as hi/lo bf16 pair (rows 64, 65)
    for bc in range(B // 512):
        cs = slice(bc * 512, (bc + 1) * 512)
        sqi = misc.tile([1, 512], F32, name=f"sqi{lidx}_{bc}", tag="z")
        for h in range(2):
            nc.tensor.matmul(sqi[:, h * 256:(h + 1) * 256], w["ones64f"][:],
                             gsqr[0:64, bc * 512 + h * 256: bc * 512 + (h + 1) * 256],
                             start=True, stop=True)
        nsq = sb.tile([1, 512], F32, name=f"nsq{lidx}_{bc}", tag="nsq")
        nc.scalar.mul(nsq[:], sqi[:], -0.5)
        hi = sb.tile([1, 512], BF16, name=f"hi{lidx}_{bc}", tag="hi")
        nc.scalar.copy(hi[:], nsq[:])
        lo = sb.tile([1, 512], F32, name=f"lo{lidx}_{bc}", tag="lo")
        nc.vector.tensor_tensor(lo[:], nsq[:], hi[:], Alu.subtract)
        lob = sb.tile([1, 512], BF16, name=f"lob{lidx}_{bc}", tag="lob")
        nc.scalar.copy(lob[:], lo[:])
        nc.sync.dma_start(aug_mov[64:65, cs], hi[:])                # hi (bf16)
        nc.sync.dma_start(aug_mov[65:66, cs], lob[:])               # lo (bf16)

    # --- diagonal correction term: corr[f, i] =
    #       sigth * gnn_f32[f, i]  -  bf16(sigth) * gnn_bf16_stored[f, i]
    corr = sb.tile([128, B], F32, name=f"corr{lidx}", tag="corr")
    for bc in range(B // 512):
        cs = slice(bc * 512, (bc + 1) * 512)
        gt = misc.tile([128, 512], F32, name=f"gt{lidx}_{bc}", tag="z")
        for h in range(2):
            nc.tensor.matmul(gt[:, h * 256:(h + 1) * 256], w["wgn_f32"][:],
                             xr_f32[:, bc * 512 + h * 256: bc * 512 + (h + 1) * 256],
                             start=True, stop=True)
        # (psum + gnn_b) * sigth  -> f32
        nc.vector.tensor_scalar(corr[:, cs], gt[:], w["wgnb"][:], w["sigthv"][:],
                                Alu.add, Alu.mult)
    # reproduce the bf16 stored gnn values for the block, transpose, subtract
    for bt in range(B // 128):
        grp, q = bt // 4, bt % 4
        if q == 0:
            gp4 = misc.tile([128, 512], F32, name=f"gp4{lidx}_{grp}", tag="z")
            st = sb.tile([128, 512], BF16, name=f"st{lidx}_{grp}", tag="st")
        nc.tensor.matmul(gp4[:, q * 128:(q + 1) * 128],
                         xr_bf[:, bt * 128:(bt + 1) * 128],
                         w["wgn_bf"][:], start=True, stop=True)
        if q == 3:
            cs = slice(grp * 512, (grp + 1) * 512)
            nc.vector.tensor_tensor(st[:], gp4[:], bcb_sb[:], Alu.add)
            for qq in range(4):
                bt2 = grp * 4 + qq
                tp = misc.tile([128, 128], BF16, name=f"tp{lidx}_{bt2}", tag="z")
                nc.tensor.transpose(tp[:], st[:, qq * 128:(qq + 1) * 128],
                                    w["ident"][:])
                st2 = sb.tile([128, 128], F32, name=f"st2{lidx}_{bt2}", tag="st2")
                nc.scalar.mul(st2[:], tp[:], w["bfsigthv"][:])
                nc.vector.tensor_tensor(corr[:, bt2 * 128:(bt2 + 1) * 128],
                                        corr[:, bt2 * 128:(bt2 + 1) * 128],
                                        st2[:], Alu.subtract)

    # --- full-N g projection into aug_g rows 0:64 (bf16), ones rows via DMA
    aug_g = sb.tile([66, N], BF16, name=f"aug_g{lidx}", tag="aug_g")
    nc.sync.dma_start(aug_g[64:66, :], w["ones2"][:, :])
    for jc in range(N // 512):
        gp = misc.tile([64, 512], F32, name=f"gp{lidx}_{jc}", tag="z")
        nc.tensor.matmul(gp[:], w["wgl_bf"][:], x_bf[:, jc * 512:(jc + 1) * 512],
                         start=True, stop=True)
        # relu(psum + gl_b) -> bf16, alternating ACT/DVE
        if jc % 2 == 0:
            nc.scalar.activation(aug_g[0:64, jc * 512:(jc + 1) * 512], gp[:],
                                 Act.Relu, bias=w["glb"][0:64, :])
        else:
            nc.vector.tensor_scalar(aug_g[0:64, jc * 512:(jc + 1) * 512], gp[:],
                                    w["glb"][0:64, :], 0.0, Alu.add, Alu.max)

    # --- sqb bias table: sqb[j_local, jt] = th - t * sq_j   (f32)
    # squares of the bf16 g values (bf16, consistent to ~0.5 ulp with the
    # PE's fp32 contraction), collapsed per j-tile by a K=64 N=1 matmul.
    gsqb = sb.tile([64, N], BF16, name=f"gsqb{lidx}", tag="gnn_t")
    for jc in range(N // 512):
        cs = slice(jc * 512, (jc + 1) * 512)
        if jc % 2 == 0:
            nc.scalar.activation(gsqb[:, cs], aug_g[0:64, cs], Act.Square)
        else:
            nc.vector.tensor_tensor(gsqb[:, cs], aug_g[0:64, cs],
                                    aug_g[0:64, cs], Alu.mult)
    sqps = misc.tile([128, 128], F32, name=f"sqps{lidx}", tag="z")
    for jt in range(JT):
        nc.tensor.matmul(sqps[:, jt:jt + 1],
                         gsqb[:, jt * 128:(jt + 1) * 128], w["ones64b"][:],
                         start=True, stop=True)
    sqb = sb.tile([128, JT], F32, name=f"sqb{lidx}", tag="sqb_sb")
    nc.vector.tensor_scalar(sqb[:], sqps[:], w["negt"][:], w["thv"][:],
                            Alu.mult, Alu.add)

    # --- gnn tiles [j, f] bf16 with bias, via bias broadcast + proj matmuls
    gnn_t = sb.tile([128, N], BF16, name=f"gnn_t{lidx}", tag="gnn_t")
    for grp in range(JT // 4):
        gp2 = misc.tile([128, 512], F32, name=f"gp2{lidx}_{grp}", tag="z")
        for q in range(4):
            jt = grp * 4 + q
            nc.tensor.matmul(gp2[:, q * 128:(q + 1) * 128],
                             x_bf[:, jt * 128:(jt + 1) * 128],
                             w["wgn_bf"][:], start=True, stop=True)
        cs = slice(grp * 512, (grp + 1) * 512)
        nc.vector.tensor_tensor(gnn_t[:, cs], gp2[:], bcb_sb[:], Alu.add)

    return dict(aug_g=aug_g, aug_mov=aug_mov, sqb=sqb, gnn_t=gnn_t, corr=corr)


def _layer_main(nc, sb, zp, mp, misc, dram, prep, w, relu, lidx):
    """Main N^2 loop + normalize for one layer.  Returns x_next [128, B] f32."""
    aug_g, aug_mov = prep["aug_g"], prep["aug_mov"]
    sqb, gnn_t, corr = prep["sqb"], prep["gnn_t"], prep["corr"]

    xn = sb.tile([128, B], F32, name=f"xn{lidx}", tag="xn", bufs=2)
    msgps = [mp.tile([128, ICH], F32, name=f"msgp{lidx}_{ic}", tag="msg")
             for ic in range(NIC)]
    # f32 row-sum accumulators (DVE), one per chunk
    raccs = [sb.tile([128, ICH], F32, name=f"racc{lidx}_{ic}", tag="racc",
                     bufs=NIC) for ic in range(NIC)]
    # jt-outer loop: one weight load of aug_g / gnn serves all NIC chunks
    for jt in range(JT):
        js = slice(jt * 128, (jt + 1) * 128)
        adjs = []
        for ic in range(NIC):
            iof = ic * ICH
            z = zp.tile([128, ICH], F32, name=f"z{lidx}_{ic}_{jt}", tag="z")
            for h in range(ICH // 512):
                nc.tensor.matmul(z[:, h * 512:(h + 1) * 512], aug_g[:, js],
                                 aug_mov[:, iof + h * 512: iof + (h + 1) * 512],
                                 start=True, stop=True)
            adj = sb.tile([128, ICH], BF16, name=f"adj{lidx}_{ic}_{jt}",
                          tag="adj", bufs=2 * NIC)
            nc.scalar.activation(adj[:], z[:], Act.Sigmoid,
                                 bias=sqb[:, jt:jt + 1], scale=1.0)
            adjs.append(adj)
        for ic in range(NIC):
            adj = adjs[ic]
            for h in range(ICH // 512):
                hs = slice(h * 512, (h + 1) * 512)
                nc.tensor.matmul(msgps[ic][:, hs], gnn_t[:, js], adj[:, hs],
                                 start=(jt == 0), stop=(jt == JT - 1))
            if jt == 0:
                nc.vector.tensor_copy(raccs[ic][:], adj[:])
            else:
                nc.vector.tensor_tensor(raccs[ic][:], raccs[ic][:], adj[:],
                                        Alu.add)

    for ic in range(NIC):
        iof = ic * ICH
        # collapse the 128 partitions of racc with a ones matmul (f32)
        rsum = sb.tile([1, ICH], F32, name=f"rsum{lidx}_{ic}", tag="rsum",
                       bufs=2)
        for h in range(ICH // 256):
            hs = slice(h * 256, (h + 1) * 256)
            rs = misc.tile([1, 256], F32, name=f"rs{lidx}_{ic}_{h}", tag="z")
            nc.tensor.matmul(rs[:], w["ones128f"][:], raccs[ic][:, hs],
                             start=True, stop=True)
            nc.vector.tensor_copy(rsum[0:1, hs], rs[:])
        rcp = sb.tile([1, ICH], F32, name=f"rcp{lidx}_{ic}", tag="rcp")
        nc.vector.reciprocal(rcp[:], rsum[0:1, :])

        # normalize: xn = [relu] ((msg + corr) * rcp_broadcast)
        for h in range(ICH // 512):
            hs512 = slice(h * 512, (h + 1) * 512)
            cs = slice(iof + h * 512, iof + (h + 1) * 512)
            bc = misc.tile([128, 512], F32, name=f"bc{lidx}_{ic}_{h}", tag="z")
            for q in range(2):
                nc.tensor.matmul(bc[:, q * 256:(q + 1) * 256], w["ones1f"][:],
                                 rcp[0:1, h * 512 + q * 256: h * 512 + (q + 1) * 256],
                                 start=True, stop=True)
            nc.vector.tensor_tensor(xn[:, cs], msgps[ic][:, hs512], corr[:, cs],
                                    Alu.add)
            nc.vector.tensor_tensor(xn[:, cs], xn[:, cs], bc[:], Alu.mult)
            if relu:
                nc.vector.tensor_scalar(xn[:, cs], xn[:, cs], 0.0, None, Alu.max)
    return xn


def build():
    nc = bacc.Bacc("TRN2", target_bir_lowering=False, debug=False,
                   num_devices=NCORES)

    ins = {}

    def di(name, shape, dt):
        ins[name] = nc.dram_tensor(name, shape, dt, kind="ExternalInput")
        return ins[name]

    di("x_bf", [D, N], BF16)
    di("xr_bf", [D, B], BF16)
    di("xr_f32", [D, B], F32)
    di("ident", [128, 128], BF16)
    di("identf", [128, 128], F32)
    di("ones2", [2, N], BF16)
    for l in range(2):
        di(f"wgl{l}", [D, L], BF16)
        di(f"glb{l}", [L, 1], F32)
        di(f"wgn{l}", [D, D], BF16)
        di(f"wgn32_{l}", [D, D], F32)
        di(f"wgnb{l}", [D, 1], F32)
        di(f"gnnbrow{l}", [1, D], F32)
    di("out_w", [D, NOUT], F32)
    di("out_b", [1, NOUT], F32)
    for nm in ("negt", "thv", "twot", "sigthv", "bfsigthv"):
        di(nm, [128, 1], F32)
    y_ext = nc.dram_tensor("y", [B, NOUT], F32, kind="ExternalOutput")

    with tile.TileContext(nc) as tc:
        with (
            tc.tile_pool(name="sb", bufs=1) as sb,
            tc.tile_pool(name="sbl", bufs=2) as sbl,       # small loop tiles
            tc.tile_pool(name="zp", bufs=2, space="PSUM") as zp,
            tc.tile_pool(name="mp", bufs=2, space="PSUM") as mp,
            tc.tile_pool(name="dram", bufs=1, space="DRAM") as dram,
        ):
            # ---- load shared small tensors
            def ld(name, shape, dt, pool=sb):
                t = pool.tile(shape, dt, name=f"{name}_sb")
                nc.sync.dma_start(t[:], ins[name][:, :])
                return t

            wsh = {}
            wsh["ident"] = ld("ident", [128, 128], BF16)
            wsh["identf"] = ld("identf", [128, 128], F32)
            for nm in ("negt", "thv", "twot", "sigthv", "bfsigthv"):
                wsh[nm] = ld(nm, [128, 1], F32)
            out_w_sb = ld("out_w", [D, NOUT], F32)
            out_b_sb = ld("out_b", [1, NOUT], F32)
            ones64f = sb.tile([64, 1], F32, name="ones64f")
            nc.vector.memset(ones64f[:], 1.0)
            ones1f = sb.tile([1, 128], F32, name="ones1f")
            nc.vector.memset(ones1f[:], 1.0)
            ones64b = sb.tile([64, 1], BF16, name="ones64b")
            nc.vector.memset(ones64b[:], 1.0)
            ones128f = sb.tile([128, 1], F32, name="ones128f")
            nc.vector.memset(ones128f[:], 1.0)
            wsh["ones64f"] = ones64f
            wsh["ones1f"] = ones1f
            wsh["ones64b"] = ones64b
            wsh["ones128f"] = ones128f
            wsh["ones2"] = ins["ones2"]

            wl = []
            for l in range(2):
                wd = dict(wsh)
                wd["wgl_bf"] = ld(f"wgl{l}", [D, L], BF16)
                glb = sb.tile([64, 1], F32, name=f"glb{l}_sb")
                nc.sync.dma_start(glb[:], ins[f"glb{l}"][:, :])
                wd["glb"] = glb
                wd["wgn_bf"] = ld(f"wgn{l}", [D, D], BF16)
                wd["wgn_f32"] = ld(f"wgn32_{l}", [D, D], F32)
                wd["wgnb"] = ld(f"wgnb{l}", [D, 1], F32)
                wd["gnnbrow"] = ld(f"gnnbrow{l}", [1, D], F32)
                wl.append(wd)

            # ---- layer 1 activations from host
            x_bf0 = sb.tile([D, N], BF16, name="x_bf0", tag="x_bf")
            for r in range(8):
                nc.sync.dma_start(x_bf0[:, r * (N // 8):(r + 1) * (N // 8)],
                                  ins["x_bf"][:, r * (N // 8):(r + 1) * (N // 8)])
            xr_bf0 = sb.tile([D, B], BF16, name="xr_bf0", tag="xr_bf")
            nc.sync.dma_start(xr_bf0[:], ins["xr_bf"][:, :])
            xr_f0 = sb.tile([D, B], F32, name="xr_f0", tag="xr_f")
            nc.sync.dma_start(xr_f0[:], ins["xr_f32"][:, :])

            # ---- layer 1
            prep0 = _layer_prep(nc, sb, zp, x_bf0, xr_bf0, xr_f0, wl[0], 0)
            x1 = _layer_main(nc, sb, zp, mp, zp, dram, prep0, wl[0], True, 0)

            # ---- AllGather x1 (bf16)
            x1_bf = sb.tile([D, B], BF16, name="x1_bf", tag="xr_bf")
            nc.vector.tensor_copy(x1_bf[:], x1[:])
            ag_in = dram.tile([D, B], BF16, name="ag_in")
            ag_out = dram.tile([NCORES * D, B], BF16, name="ag_out",
                               addr_space="Shared")
            nc.sync.dma_start(ag_in[:], x1_bf[:])
            nc.gpsimd.collective_compute(
                "AllGather", Alu.bypass,
                ins=[ag_in.opt()],
                outs=[ag_out.opt()],
                replica_groups=[list(range(NCORES))],
            )
            x_bf1 = sb.tile([D, N], BF16, name="x_bf1", tag="x_bf")
            for r in range(NCORES):
                nc.sync.dma_start(x_bf1[:, r * B:(r + 1) * B],
                                  ag_out[r * D:(r + 1) * D, :])
            # ---- layer 2
            prep1 = _layer_prep(nc, sb, zp, x_bf1, x1_bf, x1, wl[1], 1)
            x2 = _layer_main(nc, sb, zp, mp, zp, dram, prep1, wl[1], False, 1)

            # ---- output head: softmax(x2 @ out_w + out_b), 4 row-tiles
            # per PSUM group, exp without max-shift (logits are O(1))
            for grp in range(B // 512):
                lg = zp.tile([128, 4 * NOUT], F32, name=f"lg{grp}", tag="z")
                for q in range(4):
                    it = grp * 4 + q
                    qs = slice(q * NOUT, (q + 1) * NOUT)
                    nc.tensor.matmul(lg[:, qs], ones1f[:], out_b_sb[:, :],
                                     start=True, stop=False)
                    nc.tensor.matmul(lg[:, qs], x2[:, it * 128:(it + 1) * 128],
                                     out_w_sb[:], start=False, stop=True)
                e = sbl.tile([128, 4 * NOUT], F32, name=f"e{grp}", tag="e")
                nc.scalar.activation(e[:], lg[:], Act.Exp)
                e3 = e[:].rearrange("p (q n) -> p q n", n=NOUT)
                es = sbl.tile([128, 4], F32, name=f"es{grp}", tag="es")
                nc.vector.reduce_sum(es[:], e3, axis=AX)
                rse = sbl.tile([128, 4], F32, name=f"rse{grp}", tag="rse")
                nc.vector.reciprocal(rse[:], es[:])
                yt = sbl.tile([128, 4 * NOUT], F32, name=f"yt{grp}", tag="yt")
                nc.vector.tensor_tensor(
                    yt[:].rearrange("p (q n) -> p q n", n=NOUT), e3,
                    rse[:].rearrange("p q -> p q ()").broadcast_to([128, 4, NOUT]),
                    Alu.mult)
                for q in range(4):
                    it = grp * 4 + q
                    nc.sync.dma_start(y_ext[it * 128:(it + 1) * 128, :],
                                      yt[:, q * NOUT:(q + 1) * NOUT])

    nc.compile()
    return nc


def _get_nc():
    if "nc" not in _NC_CACHE:
        _NC_CACHE["nc"] = build()
    return _NC_CACHE["nc"]


def kernel(feat_matrix, gl_w0, gl_b0, gl_w1, gl_b1,
           gnn_w0, gnn_b0, gnn_w1, gnn_b1,
           out_w, out_b, temp, theta,
           adj_matrix=None, get_item_index=None, set_index=None,
           val_index=None, mask_matrix=None, **_unused):
    bf = ml_dtypes.bfloat16
    f32 = np.float32

    x = np.ascontiguousarray(np.asarray(feat_matrix, dtype=f32))
    assert x.shape == (N, D)
    t = 1.0 + float(np.asarray(temp))
    th = 5.0 + float(np.asarray(theta))
    sigth = float(1.0 / (1.0 + np.exp(-np.float32(th))))
    bfsigth = float(np.float32(bf(np.float32(sigth))))
    # ensure the computed diagonal can't straddle a bf16 rounding boundary
    lo16 = float(np.float32(bf(np.nextafter(np.float32(sigth), np.float32(0.0)))))
    hi16 = float(np.float32(bf(np.nextafter(np.float32(sigth), np.float32(1.0)))))
    assert lo16 == bfsigth == hi16, "sigth too close to a bf16 boundary"

    xT = np.ascontiguousarray(x.T)                       # [D, N] f32
    xT_bf = xT.astype(bf)

    def colvec(v):
        return np.full((128, 1), v, dtype=f32)

    common = {
        "x_bf": xT_bf,
        "ident": np.eye(128, dtype=bf),
        "identf": np.eye(128, dtype=f32),
        "ones2": np.ones((2, N), dtype=bf),
        "out_w": np.ascontiguousarray(np.asarray(out_w, dtype=f32)),
        "out_b": np.asarray(out_b, dtype=f32).reshape(1, NOUT),
        "negt": colvec(-t),
        "thv": colvec(th),
        "twot": colvec(2.0 * t),
        "sigthv": colvec(sigth),
        "bfsigthv": colvec(bfsigth),
    }
    for l, (wgl, glb, wgn, gnb) in enumerate(
            [(gl_w0, gl_b0, gnn_w0, gnn_b0), (gl_w1, gl_b1, gnn_w1, gnn_b1)]):
        wgl = np.ascontiguousarray(np.asarray(wgl, dtype=f32))
        wgn = np.ascontiguousarray(np.asarray(wgn, dtype=f32))
        common[f"wgl{l}"] = wgl.astype(bf)
        common[f"glb{l}"] = np.asarray(glb, dtype=f32).reshape(L, 1)
        common[f"wgn{l}"] = wgn.astype(bf)
        common[f"wgn32_{l}"] = wgn
        common[f"wgnb{l}"] = np.asarray(gnb, dtype=f32).reshape(D, 1)
        common[f"gnnbrow{l}"] = np.asarray(gnb, dtype=f32).reshape(1, D)

    in_maps = []
    for c in range(NCORES):
        blk = slice(c * B, (c + 1) * B)
        m = dict(common)
        m["xr_bf"] = np.ascontiguousarray(xT_bf[:, blk])
        m["xr_f32"] = np.ascontiguousarray(xT[:, blk])
        in_maps.append(m)

    nc = _get_nc()
    res = run_bass_kernel_spmd(nc, in_maps, core_ids=list(range(NCORES)))
    return np.concatenate([res.results[c]["y"] for c in range(NCORES)], axis=0)


if __name__ == "__main__":
    import time
    t0 = time.time()
    nc = build()
    print(f"build+compile: {time.time() - t0:.1f}s")

